# revision 7
# baseline (speedup 1.0000x reference)
"""GraphSAGE (2-layer, mean aggregation) on 8 Trainium2 NeuronCores.

Strategy (per spec sharding_hint): destination nodes are sharded across the
8 cores (49 tiles of 128 nodes per core, LPT-balanced by degree so every
tile has nearly equal incoming-edge count). Edge lists are partitioned by
destination tile and padded to a uniform chunk count T per tile so one SPMD
program serves all cores. x and (between layers) h are replicated to every
core's HBM; per-edge source rows are fetched with indirect DMA gathers of
128 rows per instruction. The segment sum for a destination tile is built
on the PE: for each 128-edge chunk a 0/1 selection matrix S[e, n] =
(dst_slot[e] == n) is formed on the vector engine (iota + is_equal) and
S^T @ messages accumulates into PSUM over the tile's chunks. The mean
division, dense lin_l/lin_r matmuls, bias and ReLU all happen on-device;
layer-1 output h round-trips through the host (re-replication only, no
host float math on the compute path) and feeds the identical layer-2
program. All float tensor computation runs on the NeuronCores; the host
only does integer index preprocessing, sharding/layout, and un-sharding.
"""
import heapq
import sys
from contextlib import ExitStack

import numpy as np

for _p in ("/opt/trn_rl_repo",):
    if _p not in sys.path:
        sys.path.insert(0, _p)

import concourse.bass as bass
import concourse.tile as tile
from concourse import bacc, mybir
from concourse.bass_utils import run_bass_kernel_spmd
from concourse.masks import make_identity

N_NODES = 50000
N_EDGES = 800000
DIM_IN, DIM_H, DIM_OUT = 128, 256, 64
N_CORES = 8
P = 128
TILES_PER_CORE = 49                      # ceil(50000 / 8 / 128)
N_TILES = N_CORES * TILES_PER_CORE       # 392
NPAD_CORE = TILES_PER_CORE * P           # 6272
PAD_SLOT = 200.0                         # dst_rel sentinel: matches no iota lane

LAST_RESULTS = []   # test harness reads profiling results from here


def _partition_nodes(deg):
    """LPT-pack nodes into N_TILES bins of <=128 nodes, minimizing max bin
    degree-sum. Returns (tile_of, slot_of, T) with T = uniform chunks/tile."""
    order = np.argsort(-deg, kind="stable")
    heap = [(0, t) for t in range(N_TILES)]
    heapq.heapify(heap)
    counts = np.zeros(N_TILES, np.int64)
    sums = np.zeros(N_TILES, np.int64)
    tile_of = np.empty(N_NODES, np.int64)
    slot_of = np.empty(N_NODES, np.int64)
    for node in order:
        while True:
            s, t = heapq.heappop(heap)
            if counts[t] < P:
                break
        tile_of[node] = t
        slot_of[node] = counts[t]
        counts[t] += 1
        sums[t] += deg[node]
        if counts[t] < P:
            heapq.heappush(heap, (sums[t], t))
    T = int(np.ceil(sums.max() / P))
    return tile_of, slot_of, T


def _build_edge_layout(src, dst, tile_of, slot_of, T):
    """Per-core chunk-major index arrays.

    Returns src_cols, dst_cols: lists (per core) of [P, 49*T] arrays where
    column t*T + j holds chunk j of tile t: lane p is edge j*128+p of that
    tile's padded edge list (src node id / dst slot, PAD entries src=0,
    dst_rel=PAD_SLOT).
    """
    etile = tile_of[dst]
    order = np.argsort(etile, kind="stable")
    counts = np.bincount(etile, minlength=N_TILES)
    src_pad = np.zeros((N_TILES, T * P), np.int64)
    dst_pad = np.full((N_TILES, T * P), PAD_SLOT, np.float32)
    rank = np.arange(N_EDGES) - np.repeat(np.concatenate([[0], np.cumsum(counts)[:-1]]), counts)
    es, ed = src[order], dst[order]
    src_pad[etile[order], rank] = es
    dst_pad[etile[order], rank] = slot_of[ed]
    src_cols, dst_cols = [], []
    for c in range(N_CORES):
        sl = slice(c * TILES_PER_CORE, (c + 1) * TILES_PER_CORE)
        s = src_pad[sl].reshape(TILES_PER_CORE, T, P).transpose(2, 0, 1).reshape(P, TILES_PER_CORE * T)
        d = dst_pad[sl].reshape(TILES_PER_CORE, T, P).transpose(2, 0, 1).reshape(P, TILES_PER_CORE * T)
        src_cols.append(np.ascontiguousarray(s))
        dst_cols.append(np.ascontiguousarray(d))
    return src_cols, dst_cols


def _build_layer_program(T, n_table, f_in, f_out, relu):
    """One SAGE layer as an SPMD bass program.

    Inputs (per core): table [n_table, f_in] (gather source, replicated),
    selfT [f_in, NPAD_CORE] (own nodes' features, transposed),
    wlT/wrT packed [128, (f_in/128)*f_out], b_col [128, ceil(f_out/128)],
    src_idx int32 [P, 49*T], dst_rel f32 [P, 49*T], deg_col [P, 49].
    Output: outT [f_out, NPAD_CORE].
    """
    SI = f_in // P                       # contraction splits (1 or 2)
    SO = (f_out + P - 1) // P            # output-partition splits
    fo_sz = min(f_out, P)
    NCH = TILES_PER_CORE * T

    nc = bacc.Bacc("TRN2", target_bir_lowering=False, debug=False,
                   enable_asserts=False, num_devices=N_CORES)
    dt = mybir.dt
    table = nc.dram_tensor("table", [n_table, f_in], dt.float32, kind="ExternalInput").ap()
    selfTs = [nc.dram_tensor(f"selfT{si}", [P, NPAD_CORE], dt.float32, kind="ExternalInput").ap()
              for si in range(SI)]
    wlT = nc.dram_tensor("wlT", [P, SI * f_out], dt.float32, kind="ExternalInput").ap()
    wrT = nc.dram_tensor("wrT", [P, SI * f_out], dt.float32, kind="ExternalInput").ap()
    b_col = nc.dram_tensor("b_col", [P, SO], dt.float32, kind="ExternalInput").ap()
    src_idx = nc.dram_tensor("src_idx", [P, NCH], dt.int32, kind="ExternalInput").ap()
    dst_rel = nc.dram_tensor("dst_rel", [P, NCH], dt.float32, kind="ExternalInput").ap()
    deg_col = nc.dram_tensor("deg_col", [P, TILES_PER_CORE], dt.float32, kind="ExternalInput").ap()
    outT = nc.dram_tensor("outT", [f_out, NPAD_CORE], dt.float32, kind="ExternalOutput").ap()

    with tile.TileContext(nc) as tc:
        with ExitStack() as ctx:
            const = ctx.enter_context(tc.tile_pool(name="const", bufs=1))
            msgp = ctx.enter_context(tc.tile_pool(name="msgp", bufs=2))
            sp = ctx.enter_context(tc.tile_pool(name="sp", bufs=2))
            work = ctx.enter_context(tc.tile_pool(name="work", bufs=2))
            outp = ctx.enter_context(tc.tile_pool(name="outp", bufs=3))
            psA = ctx.enter_context(tc.tile_pool(name="psA", bufs=2, space="PSUM"))
            psB = ctx.enter_context(tc.tile_pool(name="psB", bufs=2, space="PSUM"))
            psC = ctx.enter_context(tc.tile_pool(name="psC", bufs=2, space="PSUM"))

            idx_sb = const.tile([P, NCH], dt.int32)
            nc.sync.dma_start(idx_sb[:], src_idx[:, :])
            dr_sb = const.tile([P, NCH], dt.float32)
            nc.sync.dma_start(dr_sb[:], dst_rel[:, :])
            deg_sb = const.tile([P, TILES_PER_CORE], dt.float32)
            nc.sync.dma_start(deg_sb[:], deg_col[:, :])
            wl_sb = const.tile([P, SI * f_out], dt.float32)
            nc.sync.dma_start(wl_sb[:], wlT[:, :])
            wr_sb = const.tile([P, SI * f_out], dt.float32)
            nc.sync.dma_start(wr_sb[:], wrT[:, :])
            b_sb = const.tile([P, SO], dt.float32)
            nc.sync.dma_start(b_sb[:], b_col[:, :])
            self_sb = []
            for si in range(SI):
                t_ = const.tile([P, NPAD_CORE], dt.float32, name=f"self_sb{si}")
                nc.sync.dma_start(t_[:], selfTs[si][:, :])
                self_sb.append(t_)

            ident = const.tile([P, P], dt.float32)
            make_identity(nc, ident[:])
            iota_sm = const.tile([P, P], dt.float32)
            nc.gpsimd.iota(iota_sm[:], pattern=[[1, P]], base=0, channel_multiplier=0,
                           allow_small_or_imprecise_dtypes=True)
            iota_big = const.tile([P, T * P], dt.float32)
            for _j in range(T):
                nc.vector.tensor_copy(iota_big[:, _j * P:(_j + 1) * P], iota_sm[:])

            recip = const.tile([P, TILES_PER_CORE], dt.float32)
            nc.vector.tensor_scalar_max(recip[:], deg_sb[:], 1.0)
            nc.vector.reciprocal(recip[:], recip[:])

            for t in range(TILES_PER_CORE):
                c0 = t * T
                # gather the tile's T*128 messages, 128 rows per instruction
                msgs = msgp.tile([P, T * f_in], dt.float32)
                for j in range(T):
                    nc.gpsimd.indirect_dma_start(
                        out=msgs[:, j * f_in:(j + 1) * f_in],
                        out_offset=None,
                        in_=table[:, :],
                        in_offset=bass.IndirectOffsetOnAxis(ap=idx_sb[:, c0 + j:c0 + j + 1], axis=0),
                    )
                # selection matrices for all T chunks in one vector op
                S = sp.tile([P, T * P], dt.float32)
                try:
                    nc.vector.tensor_tensor(
                        out=S[:],
                        in0=dr_sb[:, c0:c0 + T, None].to_broadcast([P, T, P]),
                        in1=iota_big[:],
                        op=mybir.AluOpType.is_equal,
                    )
                except Exception:
                    for j in range(T):
                        nc.vector.tensor_tensor(
                            out=S[:, j * P:(j + 1) * P],
                            in0=dr_sb[:, c0 + j:c0 + j + 1].to_broadcast([P, P]),
                            in1=iota_big[:, :P],
                            op=mybir.AluOpType.is_equal,
                        )
                # segment sum: agg[n, f] += S_j^T @ msgs_j
                agg_ps = psA.tile([P, f_in], dt.float32)
                for j in range(T):
                    nc.tensor.matmul(
                        out=agg_ps[:],
                        lhsT=S[:, j * P:(j + 1) * P],
                        rhs=msgs[:, j * f_in:(j + 1) * f_in],
                        start=(j == 0),
                        stop=(j == T - 1),
                    )
                # mean: scale by 1/deg (per-partition scalar), PSUM -> SBUF
                agg_sb = work.tile([P, f_in], dt.float32)
                nc.scalar.mul(agg_sb[:], agg_ps[:], recip[:, t:t + 1])
                # transpose to [f_in, nodes]
                aggT_sb = []
                for si in range(SI):
                    tp = psB.tile([P, P], dt.float32)
                    nc.tensor.transpose(out=tp[:], in_=agg_sb[:, si * P:(si + 1) * P], identity=ident[:])
                    ts = work.tile([P, P], dt.float32)
                    nc.vector.tensor_copy(ts[:], tp[:])
                    aggT_sb.append(ts)
                # dense: zT[fo,n] = sum_si wlT_si^T @ aggT_si + wrT_si^T @ selfT_si
                for so in range(SO):
                    z_ps_full = psC.tile([P, P], dt.float32)
                    z_ps = z_ps_full[:fo_sz, :]
                    nmm = 2 * SI
                    k = 0
                    for si in range(SI):
                        nc.tensor.matmul(
                            out=z_ps[:],
                            lhsT=wl_sb[:, si * f_out + so * fo_sz: si * f_out + so * fo_sz + fo_sz],
                            rhs=aggT_sb[si][:],
                            start=(k == 0), stop=(k == nmm - 1))
                        k += 1
                    for si in range(SI):
                        nc.tensor.matmul(
                            out=z_ps[:],
                            lhsT=wr_sb[:, si * f_out + so * fo_sz: si * f_out + so * fo_sz + fo_sz],
                            rhs=self_sb[si][:, t * P:(t + 1) * P],
                            start=(k == 0), stop=(k == nmm - 1))
                        k += 1
                    o_sb_full = outp.tile([P, P], dt.float32)
                    o_sb = o_sb_full[:fo_sz, :]
                    if relu:
                        nc.scalar.activation(o_sb[:], z_ps[:], mybir.ActivationFunctionType.Relu,
                                             bias=b_sb[:fo_sz, so:so + 1], scale=1.0)
                    else:
                        nc.vector.tensor_add(o_sb[:], z_ps[:], b_sb[:fo_sz, so:so + 1].to_broadcast([fo_sz, P]))
                    nc.sync.dma_start(outT[so * P:so * P + fo_sz, t * P:(t + 1) * P], o_sb[:])
    nc.compile()
    return nc


_PROG_CACHE = {}


def _get_programs(T):
    key = T
    if key not in _PROG_CACHE:
        l1 = _build_layer_program(T, N_NODES, DIM_IN, DIM_H, relu=True)
        l2 = _build_layer_program(T, N_CORES * NPAD_CORE, DIM_H, DIM_OUT, relu=False)
        _PROG_CACHE[key] = (l1, l2)
    return _PROG_CACHE[key]


def _pack_w(w):
    """[f_out, f_in] weight -> [128, SI*f_out] with [p, si*f_out+f] = w[f, si*128+p]."""
    f_out, f_in = w.shape
    si = f_in // P
    return np.ascontiguousarray(np.hstack([w.T[i * P:(i + 1) * P, :] for i in range(si)]), dtype=np.float32)


def _pack_b(b):
    so = (b.shape[0] + P - 1) // P
    out = np.zeros((P, so), np.float32)
    for i in range(so):
        seg = b[i * P:(i + 1) * P]
        out[:seg.shape[0], i] = seg
    return out


def kernel(x, edge_index, W1l, W1r, b1, W2l, W2r, b2):
    global LAST_RESULTS
    LAST_RESULTS = []
    x = np.asarray(x, np.float32)
    src = np.asarray(edge_index[0], np.int64)
    dst = np.asarray(edge_index[1], np.int64)

    deg = np.bincount(dst, minlength=N_NODES)
    tile_of, slot_of, T = _partition_nodes(deg)
    src_cols, dst_cols = _build_edge_layout(src, dst, tile_of, slot_of, T)

    pos_of = tile_of * P + slot_of        # global padded slot (core = tile//49)
    l1, l2 = _get_programs(T)

    trace = bool(int(__import__("os").environ.get("BASS_TRACE", "0") or 0))
    tkw = dict(trace=True, tmpdir=None) if trace else {}

    # per-core metadata
    deg_cols, selfTs = [], []
    for c in range(N_CORES):
        sl = slice(c * TILES_PER_CORE, (c + 1) * TILES_PER_CORE)
        dcol = np.zeros((P, TILES_PER_CORE), np.float32)
        sT = np.zeros((NPAD_CORE, DIM_IN), np.float32)
        tiles = np.arange(*sl.indices(N_TILES)[:2])
        mask = np.isin(tile_of, tiles)
        nodes = np.nonzero(mask)[0]
        local = (tile_of[nodes] - c * TILES_PER_CORE) * P + slot_of[nodes]
        dcol[slot_of[nodes], tile_of[nodes] - c * TILES_PER_CORE] = deg[nodes]
        sT[local] = x[nodes]
        deg_cols.append(dcol)
        selfTs.append(np.ascontiguousarray(sT.T))

    w1l_p, w1r_p, b1_p = _pack_w(np.asarray(W1l)), _pack_w(np.asarray(W1r)), _pack_b(np.asarray(b1))
    w2l_p, w2r_p, b2_p = _pack_w(np.asarray(W2l)), _pack_w(np.asarray(W2r)), _pack_b(np.asarray(b2))

    in_maps = []
    for c in range(N_CORES):
        in_maps.append({
            "table": x,
            "selfT0": selfTs[c],
            "wlT": w1l_p, "wrT": w1r_p, "b_col": b1_p,
            "src_idx": src_cols[c].astype(np.int32),
            "dst_rel": dst_cols[c],
            "deg_col": deg_cols[c],
        })
    r1 = run_bass_kernel_spmd(l1, in_maps, core_ids=list(range(N_CORES)), **tkw)
    LAST_RESULTS.append(r1)

    # assemble full h (replicated gather table for layer 2) and per-core selfT
    h_table = np.concatenate([np.ascontiguousarray(r1.results[c]["outT"].T)
                              for c in range(N_CORES)], axis=0)  # [50176, 256]

    src2 = pos_of[src].astype(np.int32)
    src2_cols = []
    for c in range(N_CORES):
        sc = src_cols[c].copy()
        pad = dst_cols[c] == PAD_SLOT
        sc2 = pos_of[sc]
        sc2[pad] = 0
        src2_cols.append(sc2.astype(np.int32))

    in_maps2 = []
    for c in range(N_CORES):
        hT = r1.results[c]["outT"]
        in_maps2.append({
            "table": h_table,
            "selfT0": np.ascontiguousarray(hT[:128]),
            "selfT1": np.ascontiguousarray(hT[128:]),
            "wlT": w2l_p, "wrT": w2r_p, "b_col": b2_p,
            "src_idx": src2_cols[c],
            "dst_rel": dst_cols[c],
            "deg_col": deg_cols[c],
        })
    r2 = run_bass_kernel_spmd(l2, in_maps2, core_ids=list(range(N_CORES)), **tkw)
    LAST_RESULTS.append(r2)

    big = np.concatenate([r2.results[c]["outT"] for c in range(N_CORES)], axis=1)  # [64, 50176]
    out = np.ascontiguousarray(big[:, pos_of[np.arange(N_NODES)]].T, dtype=np.float32)
    return out


# revision 9
# speedup vs baseline: 1.0015x; 1.0015x over previous
"""GraphSAGE (2-layer, mean aggregation) on 8 Trainium2 NeuronCores.

Strategy (per spec sharding_hint): destination nodes are sharded across the
8 cores (49 tiles of 128 nodes per core, LPT-balanced by degree so every
tile has nearly equal incoming-edge count). Edge lists are partitioned by
destination tile and padded to a uniform chunk count T per tile so one SPMD
program serves all cores. x and (between layers) h are replicated to every
core's HBM; per-edge source rows are fetched with indirect DMA gathers of
128 rows per instruction. The segment sum for a destination tile is built
on the PE: for each 128-edge chunk a 0/1 selection matrix S[e, n] =
(dst_slot[e] == n) is formed on the vector engine (iota + is_equal) and
S^T @ messages accumulates into PSUM over the tile's chunks. The mean
division, dense lin_l/lin_r matmuls, bias and ReLU all happen on-device;
layer-1 output h round-trips through the host (re-replication only, no
host float math on the compute path) and feeds the identical layer-2
program. All float tensor computation runs on the NeuronCores; the host
only does integer index preprocessing, sharding/layout, and un-sharding.
"""
import heapq
import sys
from contextlib import ExitStack

import numpy as np

for _p in ("/opt/trn_rl_repo",):
    if _p not in sys.path:
        sys.path.insert(0, _p)

import concourse.bass as bass
import concourse.tile as tile
from concourse import bacc, mybir
from concourse.bass_utils import run_bass_kernel_spmd
from concourse.masks import make_identity


def _ensure_axon_hooks():
    """run_bass_kernel_spmd(trace=True) imports antenv.axon_hooks, which this
    image lacks; install a ctypes-backed hook so tracing works (or degrades
    to a no-op instead of an ImportError)."""
    try:
        import antenv.axon_hooks  # noqa: F401
        return
    except ImportError:
        pass
    import contextlib
    import ctypes
    import types

    def _make_hook():
        try:
            lib = ctypes.CDLL("/opt/axon/libaxon_pjrt.so")
        except OSError:
            return None
        if not hasattr(lib, "axon_start_nrt_profile"):
            return None
        lib.axon_start_nrt_profile.argtypes = [ctypes.POINTER(ctypes.c_int64), ctypes.c_size_t]
        lib.axon_start_nrt_profile.restype = ctypes.c_int64
        lib.axon_stop_nrt_profile.argtypes = [ctypes.c_char_p]
        lib.axon_stop_nrt_profile.restype = ctypes.c_int64

        @contextlib.contextmanager
        def _hook(output_dir, device_ids):
            import jax
            jax.devices()
            if device_ids:
                ids = (ctypes.c_int64 * len(device_ids))(*device_ids)
                rc = lib.axon_start_nrt_profile(ids, len(device_ids))
            else:
                rc = lib.axon_start_nrt_profile(None, 0)
            if rc != 0:
                raise RuntimeError(f"axon_start_nrt_profile rc={rc}")
            try:
                yield
            finally:
                lib.axon_stop_nrt_profile(str(output_dir).encode())

        return _hook

    hook = _make_hook()
    mod = types.ModuleType("antenv.axon_hooks")
    mod.get_axon_ntff_profile_hook = lambda: hook
    mod.set_axon_ntff_profile_hook = lambda h: None
    import antenv
    antenv.axon_hooks = mod
    sys.modules["antenv.axon_hooks"] = mod


_ensure_axon_hooks()


def _run_spmd_retry(nc, in_maps, **kw):
    """One retry for transient NRT device errors (axon cores occasionally
    report EXEC_UNIT_UNRECOVERABLE right after a prior faulted run)."""
    import time
    try:
        return run_bass_kernel_spmd(nc, in_maps, core_ids=list(range(N_CORES)), **kw)
    except Exception:
        time.sleep(15)
        return run_bass_kernel_spmd(nc, in_maps, core_ids=list(range(N_CORES)), **kw)

N_NODES = 50000
N_EDGES = 800000
DIM_IN, DIM_H, DIM_OUT = 128, 256, 64
N_CORES = 8
P = 128
TILES_PER_CORE = 49                      # ceil(50000 / 8 / 128)
N_TILES = N_CORES * TILES_PER_CORE       # 392
NPAD_CORE = TILES_PER_CORE * P           # 6272
PAD_SLOT = 200.0                         # dst_rel sentinel: matches no iota lane

LAST_RESULTS = []   # test harness reads profiling results from here


def _partition_nodes(deg):
    """LPT-pack nodes into N_TILES bins of <=128 nodes, minimizing max bin
    degree-sum. Returns (tile_of, slot_of, T) with T = uniform chunks/tile."""
    order = np.argsort(-deg, kind="stable")
    heap = [(0, t) for t in range(N_TILES)]
    heapq.heapify(heap)
    counts = np.zeros(N_TILES, np.int64)
    sums = np.zeros(N_TILES, np.int64)
    tile_of = np.empty(N_NODES, np.int64)
    slot_of = np.empty(N_NODES, np.int64)
    for node in order:
        while True:
            s, t = heapq.heappop(heap)
            if counts[t] < P:
                break
        tile_of[node] = t
        slot_of[node] = counts[t]
        counts[t] += 1
        sums[t] += deg[node]
        if counts[t] < P:
            heapq.heappush(heap, (sums[t], t))
    T = int(np.ceil(sums.max() / P))
    return tile_of, slot_of, T


def _build_edge_layout(src, dst, tile_of, slot_of, T):
    """Per-core chunk-major index arrays.

    Returns src_cols, dst_cols: lists (per core) of [P, 49*T] arrays where
    column t*T + j holds chunk j of tile t: lane p is edge j*128+p of that
    tile's padded edge list (src node id / dst slot, PAD entries src=0,
    dst_rel=PAD_SLOT).
    """
    etile = tile_of[dst]
    order = np.argsort(etile, kind="stable")
    counts = np.bincount(etile, minlength=N_TILES)
    src_pad = np.zeros((N_TILES, T * P), np.int64)
    dst_pad = np.full((N_TILES, T * P), PAD_SLOT, np.float32)
    rank = np.arange(N_EDGES) - np.repeat(np.concatenate([[0], np.cumsum(counts)[:-1]]), counts)
    es, ed = src[order], dst[order]
    src_pad[etile[order], rank] = es
    dst_pad[etile[order], rank] = slot_of[ed]
    src_cols, dst_cols = [], []
    for c in range(N_CORES):
        sl = slice(c * TILES_PER_CORE, (c + 1) * TILES_PER_CORE)
        s = src_pad[sl].reshape(TILES_PER_CORE, T, P).transpose(2, 0, 1).reshape(P, TILES_PER_CORE * T)
        d = dst_pad[sl].reshape(TILES_PER_CORE, T, P).transpose(2, 0, 1).reshape(P, TILES_PER_CORE * T)
        src_cols.append(np.ascontiguousarray(s))
        dst_cols.append(np.ascontiguousarray(d))
    return src_cols, dst_cols


def _build_layer_program(T, n_table, f_in, f_out, relu):
    """One SAGE layer as an SPMD bass program.

    Inputs (per core): table [n_table, f_in] (gather source, replicated),
    selfT [f_in, NPAD_CORE] (own nodes' features, transposed),
    wlT/wrT packed [128, (f_in/128)*f_out], b_col [128, ceil(f_out/128)],
    src_idx int32 [P, 49*T], dst_rel f32 [P, 49*T], deg_col [P, 49].
    Output: outT [f_out, NPAD_CORE].
    """
    SI = f_in // P                       # contraction splits (1 or 2)
    SO = (f_out + P - 1) // P            # output-partition splits
    fo_sz = min(f_out, P)
    NCH = TILES_PER_CORE * T

    nc = bacc.Bacc("TRN2", target_bir_lowering=False, debug=False,
                   enable_asserts=False, num_devices=N_CORES)
    dt = mybir.dt
    table = nc.dram_tensor("table", [n_table, f_in], dt.float32, kind="ExternalInput").ap()
    selfTs = [nc.dram_tensor(f"selfT{si}", [P, NPAD_CORE], dt.float32, kind="ExternalInput").ap()
              for si in range(SI)]
    wlT = nc.dram_tensor("wlT", [P, SI * f_out], dt.float32, kind="ExternalInput").ap()
    wrT = nc.dram_tensor("wrT", [P, SI * f_out], dt.float32, kind="ExternalInput").ap()
    b_col = nc.dram_tensor("b_col", [P, SO], dt.float32, kind="ExternalInput").ap()
    src_idx = nc.dram_tensor("src_idx", [P, NCH], dt.int32, kind="ExternalInput").ap()
    dst_rel = nc.dram_tensor("dst_rel", [P, NCH], dt.float32, kind="ExternalInput").ap()
    deg_col = nc.dram_tensor("deg_col", [P, TILES_PER_CORE], dt.float32, kind="ExternalInput").ap()
    outT = nc.dram_tensor("outT", [f_out, NPAD_CORE], dt.float32, kind="ExternalOutput").ap()

    with tile.TileContext(nc) as tc:
        with ExitStack() as ctx:
            const = ctx.enter_context(tc.tile_pool(name="const", bufs=1))
            msgp = ctx.enter_context(tc.tile_pool(name="msgp", bufs=2))
            sp = ctx.enter_context(tc.tile_pool(name="sp", bufs=2))
            work = ctx.enter_context(tc.tile_pool(name="work", bufs=2))
            outp = ctx.enter_context(tc.tile_pool(name="outp", bufs=3))
            psA = ctx.enter_context(tc.tile_pool(name="psA", bufs=2, space="PSUM"))
            psB = ctx.enter_context(tc.tile_pool(name="psB", bufs=2, space="PSUM"))
            psC = ctx.enter_context(tc.tile_pool(name="psC", bufs=2, space="PSUM"))

            idx_sb = const.tile([P, NCH], dt.int32)
            nc.sync.dma_start(idx_sb[:], src_idx[:, :])
            dr_sb = const.tile([P, NCH], dt.float32)
            nc.sync.dma_start(dr_sb[:], dst_rel[:, :])
            deg_sb = const.tile([P, TILES_PER_CORE], dt.float32)
            nc.sync.dma_start(deg_sb[:], deg_col[:, :])
            wl_sb = const.tile([P, SI * f_out], dt.float32)
            nc.sync.dma_start(wl_sb[:], wlT[:, :])
            wr_sb = const.tile([P, SI * f_out], dt.float32)
            nc.sync.dma_start(wr_sb[:], wrT[:, :])
            b_sb = const.tile([P, SO], dt.float32)
            nc.sync.dma_start(b_sb[:], b_col[:, :])
            self_sb = []
            for si in range(SI):
                t_ = const.tile([P, NPAD_CORE], dt.float32, name=f"self_sb{si}")
                nc.sync.dma_start(t_[:], selfTs[si][:, :])
                self_sb.append(t_)

            ident = const.tile([P, P], dt.float32)
            make_identity(nc, ident[:])
            iota_sm = const.tile([P, P], dt.float32)
            nc.gpsimd.iota(iota_sm[:], pattern=[[1, P]], base=0, channel_multiplier=0,
                           allow_small_or_imprecise_dtypes=True)
            iota_big = const.tile([P, T * P], dt.float32)
            for _j in range(T):
                nc.vector.tensor_copy(iota_big[:, _j * P:(_j + 1) * P], iota_sm[:])

            recip = const.tile([P, TILES_PER_CORE], dt.float32)
            nc.vector.tensor_scalar_max(recip[:], deg_sb[:], 1.0)
            nc.vector.reciprocal(recip[:], recip[:])

            for t in range(TILES_PER_CORE):
                c0 = t * T
                # gather the tile's T*128 messages, 128 rows per instruction
                msgs = msgp.tile([P, T * f_in], dt.float32)
                for j in range(T):
                    nc.gpsimd.indirect_dma_start(
                        out=msgs[:, j * f_in:(j + 1) * f_in],
                        out_offset=None,
                        in_=table[:, :],
                        in_offset=bass.IndirectOffsetOnAxis(ap=idx_sb[:, c0 + j:c0 + j + 1], axis=0),
                    )
                # selection matrices for all T chunks in one vector op
                S = sp.tile([P, T * P], dt.float32)
                try:
                    nc.vector.tensor_tensor(
                        out=S[:],
                        in0=dr_sb[:, c0:c0 + T, None].to_broadcast([P, T, P]),
                        in1=iota_big[:],
                        op=mybir.AluOpType.is_equal,
                    )
                except Exception:
                    for j in range(T):
                        nc.vector.tensor_tensor(
                            out=S[:, j * P:(j + 1) * P],
                            in0=dr_sb[:, c0 + j:c0 + j + 1].to_broadcast([P, P]),
                            in1=iota_big[:, :P],
                            op=mybir.AluOpType.is_equal,
                        )
                # segment sum: agg[n, f] += S_j^T @ msgs_j
                agg_ps = psA.tile([P, f_in], dt.float32)
                for j in range(T):
                    nc.tensor.matmul(
                        out=agg_ps[:],
                        lhsT=S[:, j * P:(j + 1) * P],
                        rhs=msgs[:, j * f_in:(j + 1) * f_in],
                        start=(j == 0),
                        stop=(j == T - 1),
                    )
                # mean: scale by 1/deg (per-partition scalar), PSUM -> SBUF
                agg_sb = work.tile([P, f_in], dt.float32)
                nc.scalar.mul(agg_sb[:], agg_ps[:], recip[:, t:t + 1])
                # transpose to [f_in, nodes]
                aggT_sb = []
                for si in range(SI):
                    tp = psB.tile([P, P], dt.float32)
                    nc.tensor.transpose(out=tp[:], in_=agg_sb[:, si * P:(si + 1) * P], identity=ident[:])
                    ts = work.tile([P, P], dt.float32)
                    nc.vector.tensor_copy(ts[:], tp[:])
                    aggT_sb.append(ts)
                # dense: zT[fo,n] = sum_si wlT_si^T @ aggT_si + wrT_si^T @ selfT_si
                for so in range(SO):
                    z_ps_full = psC.tile([P, P], dt.float32)
                    z_ps = z_ps_full[:fo_sz, :]
                    nmm = 2 * SI
                    k = 0
                    for si in range(SI):
                        nc.tensor.matmul(
                            out=z_ps[:],
                            lhsT=wl_sb[:, si * f_out + so * fo_sz: si * f_out + so * fo_sz + fo_sz],
                            rhs=aggT_sb[si][:],
                            start=(k == 0), stop=(k == nmm - 1))
                        k += 1
                    for si in range(SI):
                        nc.tensor.matmul(
                            out=z_ps[:],
                            lhsT=wr_sb[:, si * f_out + so * fo_sz: si * f_out + so * fo_sz + fo_sz],
                            rhs=self_sb[si][:, t * P:(t + 1) * P],
                            start=(k == 0), stop=(k == nmm - 1))
                        k += 1
                    o_sb_full = outp.tile([P, P], dt.float32)
                    o_sb = o_sb_full[:fo_sz, :]
                    if relu:
                        nc.scalar.activation(o_sb[:], z_ps[:], mybir.ActivationFunctionType.Relu,
                                             bias=b_sb[:fo_sz, so:so + 1], scale=1.0)
                    else:
                        nc.vector.tensor_add(o_sb[:], z_ps[:], b_sb[:fo_sz, so:so + 1].to_broadcast([fo_sz, P]))
                    nc.sync.dma_start(outT[so * P:so * P + fo_sz, t * P:(t + 1) * P], o_sb[:])
    nc.compile()
    return nc


_PROG_CACHE = {}


def _get_programs(T):
    key = T
    if key not in _PROG_CACHE:
        l1 = _build_layer_program(T, N_NODES, DIM_IN, DIM_H, relu=True)
        l2 = _build_layer_program(T, N_CORES * NPAD_CORE, DIM_H, DIM_OUT, relu=False)
        _PROG_CACHE[key] = (l1, l2)
    return _PROG_CACHE[key]


def _pack_w(w):
    """[f_out, f_in] weight -> [128, SI*f_out] with [p, si*f_out+f] = w[f, si*128+p]."""
    f_out, f_in = w.shape
    si = f_in // P
    return np.ascontiguousarray(np.hstack([w.T[i * P:(i + 1) * P, :] for i in range(si)]), dtype=np.float32)


def _pack_b(b):
    so = (b.shape[0] + P - 1) // P
    out = np.zeros((P, so), np.float32)
    for i in range(so):
        seg = b[i * P:(i + 1) * P]
        out[:seg.shape[0], i] = seg
    return out


def kernel(x, edge_index, W1l, W1r, b1, W2l, W2r, b2):
    global LAST_RESULTS
    LAST_RESULTS = []
    x = np.asarray(x, np.float32)
    src = np.asarray(edge_index[0], np.int64)
    dst = np.asarray(edge_index[1], np.int64)

    deg = np.bincount(dst, minlength=N_NODES)
    tile_of, slot_of, T = _partition_nodes(deg)
    src_cols, dst_cols = _build_edge_layout(src, dst, tile_of, slot_of, T)

    pos_of = tile_of * P + slot_of        # global padded slot (core = tile//49)
    l1, l2 = _get_programs(T)

    trace = bool(int(__import__("os").environ.get("BASS_TRACE", "0") or 0))
    tkw = dict(trace=True, tmpdir=None) if trace else {}

    # per-core metadata
    deg_cols, selfTs = [], []
    for c in range(N_CORES):
        sl = slice(c * TILES_PER_CORE, (c + 1) * TILES_PER_CORE)
        dcol = np.zeros((P, TILES_PER_CORE), np.float32)
        sT = np.zeros((NPAD_CORE, DIM_IN), np.float32)
        tiles = np.arange(*sl.indices(N_TILES)[:2])
        mask = np.isin(tile_of, tiles)
        nodes = np.nonzero(mask)[0]
        local = (tile_of[nodes] - c * TILES_PER_CORE) * P + slot_of[nodes]
        dcol[slot_of[nodes], tile_of[nodes] - c * TILES_PER_CORE] = deg[nodes]
        sT[local] = x[nodes]
        deg_cols.append(dcol)
        selfTs.append(np.ascontiguousarray(sT.T))

    w1l_p, w1r_p, b1_p = _pack_w(np.asarray(W1l)), _pack_w(np.asarray(W1r)), _pack_b(np.asarray(b1))
    w2l_p, w2r_p, b2_p = _pack_w(np.asarray(W2l)), _pack_w(np.asarray(W2r)), _pack_b(np.asarray(b2))

    in_maps = []
    for c in range(N_CORES):
        in_maps.append({
            "table": x,
            "selfT0": selfTs[c],
            "wlT": w1l_p, "wrT": w1r_p, "b_col": b1_p,
            "src_idx": src_cols[c].astype(np.int32),
            "dst_rel": dst_cols[c],
            "deg_col": deg_cols[c],
        })
    r1 = _run_spmd_retry(l1, in_maps, **tkw)
    LAST_RESULTS.append(r1)

    # assemble full h (replicated gather table for layer 2) and per-core selfT
    h_table = np.concatenate([np.ascontiguousarray(r1.results[c]["outT"].T)
                              for c in range(N_CORES)], axis=0)  # [50176, 256]

    src2 = pos_of[src].astype(np.int32)
    src2_cols = []
    for c in range(N_CORES):
        sc = src_cols[c].copy()
        pad = dst_cols[c] == PAD_SLOT
        sc2 = pos_of[sc]
        sc2[pad] = 0
        src2_cols.append(sc2.astype(np.int32))

    in_maps2 = []
    for c in range(N_CORES):
        hT = r1.results[c]["outT"]
        in_maps2.append({
            "table": h_table,
            "selfT0": np.ascontiguousarray(hT[:128]),
            "selfT1": np.ascontiguousarray(hT[128:]),
            "wlT": w2l_p, "wrT": w2r_p, "b_col": b2_p,
            "src_idx": src2_cols[c],
            "dst_rel": dst_cols[c],
            "deg_col": deg_cols[c],
        })
    r2 = _run_spmd_retry(l2, in_maps2, **tkw)
    LAST_RESULTS.append(r2)

    big = np.concatenate([r2.results[c]["outT"] for c in range(N_CORES)], axis=1)  # [64, 50176]
    out = np.ascontiguousarray(big[:, pos_of[np.arange(N_NODES)]].T, dtype=np.float32)
    return out
